# revision 8
# baseline (speedup 1.0000x reference)
"""Trainium2 Bass kernel for the Agent forward pass (3 MLPs + KDE mixture).

Device computes the three MLPs (encoder / policy / MDN) in feature-major
layout with fp32r matmuls; host does the cheap KDE tail (25 components x 3
dims per row) plus the global-gradient-norm mix, which needs a cross-shard
reduction anyway.

Self-contained: hardcodes all shapes; imports only numpy + concourse.
"""

import os

import numpy as np

import concourse.bacc as bacc
import concourse.mybir as mybir
import concourse.tile as tile
from concourse.bass_utils import run_bass_kernel_spmd

# Problem dims (hardcoded per spec)
B = 131072
NCORES = 8
BC = B // NCORES  # 16384 rows per core
NG, ADIM = 25, 3
MU = NG * ADIM  # 75
H = 1.0
NI = 0.0005
KDE_C = float((2.0 * np.pi * H**ADIM) ** (-0.5))

NB = 512  # batch columns per slot
NSLOTS = BC // NB

# fp32 storage; fp32r = single-pass PE matmul (dst must start at psum partition 0)
ACT_NP = np.float32
ACT_DT = mybir.dt.float32r

# Engine for each elementwise op: "A" = scalar/ACT, "V" = vector/DVE
ENG = {
    "relu0": "A",   # fused layer-0 relu [128, 3*NB]
    "r1e": "V", "r1m": "V", "r1p": "A",
    "r2e": "V", "r2m": "V", "r2p": "V",
    "cmu": "A",     # mu copy [75, NB]
    "cza": "A",     # z+ail copy [3, 2*NB]
}

# --- const pack column layout (one [128, NCONST] tensor) ---
_col = 0


def _take(n):
    global _col
    c = _col
    _col += n
    return c, _col


C_EW1 = _take(128)
C_EW2 = _take(128)
C_PW1 = _take(128)
C_PW2 = _take(128)
C_MW1 = _take(128)
C_MW2 = _take(128)
C_EW0 = _take(128)  # aug: rows 0-63 ew0, row 64 eb0
C_PW0 = _take(128)  # aug: rows 0-63 pw0[:64], row 64 pb0, rows 65-96 pw0[64:]
C_MW0 = _take(128)  # aug: row 64 mb0, rows 65-96 mw0
C_MW3 = _take(96)   # mw3 [128,75] padded to 96 cols (dst psum [0:96])
C_EW3 = _take(3)
C_PW3 = _take(3)
C_EB1 = _take(1)
C_EB2 = _take(1)
C_PB1 = _take(1)
C_PB2 = _take(1)
C_MB1 = _take(1)
C_MB2 = _take(1)
NCONST = _col


def _pack_consts(w):
    """Build the [128, NCONST] f32 constant pack from the input dict."""
    P = np.zeros((128, NCONST), np.float32)

    def put(cr, arr, r0=0):
        c0, c1 = cr
        a = np.asarray(arr, np.float32)
        P[r0 : r0 + a.shape[0], c0 : c0 + a.shape[1]] = a

    put(C_EW1, w["ew1"])
    put(C_EW2, w["ew2"])
    put(C_PW1, w["pw1"])
    put(C_PW2, w["pw2"])
    put(C_MW1, w["mw1"])
    put(C_MW2, w["mw2"])
    put(C_EW0, w["ew0"])  # [64,128] rows 0-63
    put(C_EW0, w["eb0"][None, :], r0=64)
    put(C_PW0, w["pw0"][0:64])  # s part rows 0-63
    put(C_PW0, w["pb0"][None, :], r0=64)
    put(C_PW0, w["pw0"][64:96], r0=65)  # g part rows 65-96
    put(C_MW0, w["mb0"][None, :], r0=64)
    put(C_MW0, w["mw0"], r0=65)  # [32,128] rows 65-96
    put(C_MW3, w["mw3"])  # [128,75] cols 0-74 of 96
    put(C_EW3, w["ew3"])
    put(C_PW3, w["pw3"])
    put(C_EB1, w["eb1"][:, None])
    put(C_EB2, w["eb2"][:, None])
    put(C_PB1, w["pb1"][:, None])
    put(C_PB2, w["pb2"][:, None])
    put(C_MB1, w["mb1"][:, None])
    put(C_MB2, w["mb2"][:, None])
    return P


def build_program():
    """Build the per-core Bass program (same SPMD program on all 8 cores)."""
    nc = bacc.Bacc("TRN2", target_bir_lowering=False, debug=False)

    sg = nc.dram_tensor("sg", [128, BC], ACT_DT, kind="ExternalInput")
    wpack = nc.dram_tensor("wpack", [128, NCONST], ACT_DT, kind="ExternalInput")
    out_mu = nc.dram_tensor("out_mu", [75, BC], mybir.dt.float32, kind="ExternalOutput")
    out_za = nc.dram_tensor("out_za", [3, 2 * BC], mybir.dt.float32, kind="ExternalOutput")

    relu = mybir.ActivationFunctionType.Relu
    copyf = mybir.ActivationFunctionType.Copy
    add_op = mybir.AluOpType.add
    max_op = mybir.AluOpType.max

    with tile.TileContext(nc) as tc:
        with (
            tc.tile_pool(name="consts", bufs=1) as consts,
            tc.tile_pool(name="ins", bufs=3) as ins,
            tc.tile_pool(name="acts", bufs=2) as acts,
            tc.tile_pool(name="outs", bufs=3) as outs,
            tc.tile_pool(name="ps", bufs=3, space="PSUM") as ps,
            tc.tile_pool(name="psz", bufs=1, space="PSUM") as psz,
            tc.tile_pool(name="ps0", bufs=1, space="PSUM") as ps0,
        ):
            W = consts.tile([128, NCONST], ACT_DT)
            nc.sync.dma_start(out=W[:], in_=wpack[:])

            def wv(cr, r0=0, r1=128):
                c0, c1 = cr
                return W[r0:r1, c0:c1]

            def bias(cr, r0=0, r1=128):
                c0, c1 = cr
                return W[r0:r1, c0:c1].bitcast(mybir.dt.float32)

            def mm(out, lhsT, rhs):
                nc.tensor.matmul(out, lhsT, rhs, start=True, stop=True)

            def relu_bias(key, out, in_, b):
                """out = relu(in_ + b), engine per ENG[key]."""
                if ENG[key] == "A":
                    nc.scalar.activation(out=out, in_=in_, func=relu, bias=b)
                else:
                    nc.vector.tensor_scalar(
                        out=out, in0=in_, scalar1=b, scalar2=0.0,
                        op0=add_op, op1=max_op,
                    )

            def copy(key, out, in_):
                if ENG[key] == "A":
                    nc.scalar.activation(out=out, in_=in_, func=copyf)
                else:
                    nc.vector.tensor_copy(out=out, in_=in_)

            for t in range(NSLOTS):
                c0 = t * NB
                sgt = ins.tile([128, NB], ACT_DT, tag="sgt")
                nc.sync.dma_start(out=sgt[:], in_=sg[:, c0 : c0 + NB])

                # ---- layer 0 (biases folded via ones-row augmentation) ----
                p0 = ps0.tile([128, 3 * NB], mybir.dt.float32, tag="p0")
                mm(p0[:, 0:NB], wv(C_EW0, 0, 65), sgt[0:65])
                mm(p0[:, NB : 2 * NB], wv(C_MW0, 64, 97), sgt[64:97])
                mm(p0[:, 2 * NB : 3 * NB], wv(C_PW0, 0, 97), sgt[0:97])
                a1 = acts.tile([128, 3 * NB], ACT_DT, tag="a1")
                if ENG["relu0"] == "A":
                    nc.scalar.activation(out=a1[:], in_=p0[:], func=relu)
                else:
                    nc.vector.tensor_scalar_max(out=a1[:], in0=p0[:], scalar1=0.0)
                a1e, a1m, a1p = a1[:, 0:NB], a1[:, NB : 2 * NB], a1[:, 2 * NB : 3 * NB]

                # ---- layers 1 and 2 ----
                p1e = ps.tile([128, NB], mybir.dt.float32, tag="pnet")
                mm(p1e[:], wv(C_EW1), a1e)
                a2e = acts.tile([128, NB], ACT_DT, tag="a2e")
                relu_bias("r1e", a2e[:], p1e[:], bias(C_EB1))

                p1m = ps.tile([128, NB], mybir.dt.float32, tag="pnet")
                mm(p1m[:], wv(C_MW1), a1m)
                a2m = acts.tile([128, NB], ACT_DT, tag="a2m")
                relu_bias("r1m", a2m[:], p1m[:], bias(C_MB1))

                p1p = ps.tile([128, NB], mybir.dt.float32, tag="pnet")
                mm(p1p[:], wv(C_PW1), a1p)
                a2p = acts.tile([128, NB], ACT_DT, tag="a2p")
                relu_bias("r1p", a2p[:], p1p[:], bias(C_PB1))

                p2e = ps.tile([128, NB], mybir.dt.float32, tag="pnet")
                mm(p2e[:], wv(C_EW2), a2e)
                a3e = acts.tile([128, NB], ACT_DT, tag="a3e")
                relu_bias("r2e", a3e[:], p2e[:], bias(C_EB2))

                p2m = ps.tile([128, NB], mybir.dt.float32, tag="pnet")
                mm(p2m[:], wv(C_MW2), a2m)
                a3m = acts.tile([128, NB], ACT_DT, tag="a3m")
                relu_bias("r2m", a3m[:], p2m[:], bias(C_MB2))

                p2p = ps.tile([128, NB], mybir.dt.float32, tag="pnet")
                mm(p2p[:], wv(C_PW2), a2p)
                a3p = acts.tile([128, NB], ACT_DT, tag="a3p")
                relu_bias("r2p", a3p[:], p2p[:], bias(C_PB2))

                # ---- layer 3 (biases added on host) ----
                pl3 = ps.tile([128, NB], mybir.dt.float32, tag="pnet")
                mm(pl3[0:96], wv(C_MW3), a3m)
                st3 = outs.tile([75, NB], mybir.dt.float32, tag="st3")
                copy("cmu", st3[:], pl3[0:75])
                nc.sync.dma_start(out=out_mu[:, c0 : c0 + NB], in_=st3[:])

                pz2 = psz.tile([128, 2 * NB], mybir.dt.float32, tag="pz2")
                mm(pz2[0:3, 0:NB], wv(C_EW3), a3e)
                mm(pz2[0:3, NB : 2 * NB], wv(C_PW3), a3p)
                sza = outs.tile([3, 2 * NB], mybir.dt.float32, tag="sza")
                copy("cza", sza[:], pz2[0:3, :])
                nc.sync.dma_start(out=out_za[:, 2 * c0 : 2 * c0 + 2 * NB], in_=sza[:])

    nc.compile()
    return nc


_NC = None
LAST_RESULTS = None  # BassKernelResults from the most recent run (for test.py)


def _get_nc():
    global _NC
    if _NC is None:
        _NC = build_program()
    return _NC


def kernel(**inputs):
    global LAST_RESULTS
    w = {k: np.asarray(v, np.float32) for k, v in inputs.items()}
    s, g = w["s"], w["g"]

    wpack = _pack_consts(w)
    in_maps = []
    for c in range(NCORES):
        r0 = c * BC
        sgT = np.zeros((128, BC), ACT_NP)
        sgT[0:64] = s[r0 : r0 + BC].T
        sgT[64] = 1.0
        sgT[65:97] = g[r0 : r0 + BC].T
        in_maps.append({"sg": np.ascontiguousarray(sgT), "wpack": wpack})

    nc = _get_nc()
    res = run_bass_kernel_spmd(
        nc,
        in_maps,
        core_ids=list(range(NCORES)),
        trace=bool(int(os.environ.get("KERNEL_TRACE", "0"))),
    )
    LAST_RESULTS = res

    mu = np.empty((B, MU), np.float32)
    z = np.empty((B, ADIM), np.float32)
    ail = np.empty((B, ADIM), np.float32)
    for c in range(NCORES):
        r0 = c * BC
        mu[r0 : r0 + BC] = res.results[c]["out_mu"].T
        za = res.results[c]["out_za"].reshape(3, NSLOTS, 2, NB)
        z[r0 : r0 + BC] = za[:, :, 0, :].reshape(3, BC).T
        ail[r0 : r0 + BC] = za[:, :, 1, :].reshape(3, BC).T

    # layer-3 biases applied on host
    mu += np.asarray(w["mb3"], np.float32)[None, :]
    z += np.asarray(w["eb3"], np.float32)[None, :]
    ail += np.asarray(w["pb3"], np.float32)[None, :]

    # ---- host KDE tail + global-norm mix ----
    diff = z[:, None, :] - mu.reshape(B, NG, ADIM)  # [B, 25, 3]
    delta = -0.5 * np.einsum("bnd,bnd->bn", diff, diff) / (H * H)
    p = KDE_C * np.exp(delta)  # [B, 25]
    rho = p.sum(axis=-1)  # [B]
    grad = -np.einsum("bn,bnd->bd", p, diff) / (H * H)
    grad = np.nan_to_num(grad, nan=0.0)
    gnorm = np.linalg.norm(grad)
    gradn = grad / gnorm * NI
    pm = np.tanh(rho * 0.002)[:, None]
    out = pm * ail + (1.0 - pm) * gradn
    return out.astype(np.float32)


# revision 10
# speedup vs baseline: 1.0609x; 1.0609x over previous
"""Trainium2 Bass kernel for the Agent forward pass (3 MLPs + KDE mixture).

Device computes the three MLPs (encoder / policy / MDN) in feature-major
layout (fp16 matmul operands, fp32 psum); host does the cheap KDE tail
(25 components x 3 dims per row) plus the global-gradient-norm mix, which
needs a cross-shard reduction anyway.

Self-contained: hardcodes all shapes; imports only numpy + concourse.
"""

import os

import numpy as np

import concourse.bacc as bacc
import concourse.mybir as mybir
import concourse.tile as tile
from concourse.bass_utils import run_bass_kernel_spmd

# Problem dims (hardcoded per spec)
B = 131072
NCORES = 8
BC = B // NCORES  # 16384 rows per core
NG, ADIM = 25, 3
MU = NG * ADIM  # 75
H = 1.0
NI = 0.0005
KDE_C = float((2.0 * np.pi * H**ADIM) ** (-0.5))

NB = 1024  # batch columns per slot
NSLOTS = BC // NB
NMM = 512  # matmul moving-operand chunk

ACT_DT = mybir.dt.float16
ACT_NP = np.float16

# Engine for each elementwise op: "A" = scalar/ACT, "V" = vector/DVE
ENG = {
    "r0e": "A", "r0m": "V", "r0p": "V",
    "r1e": "V", "r1m": "V", "r1p": "A",
    "r2e": "V", "r2m": "V", "r2p": "A",
    "cmu": "A", "cz": "A", "ca": "A",
}

# --- const pack column layout ---
_col = 0


def _take(n):
    global _col
    c = _col
    _col += n
    return c, _col


C_EW1 = _take(128)
C_EW2 = _take(128)
C_PW1 = _take(128)
C_PW2 = _take(128)
C_MW1 = _take(128)
C_MW2 = _take(128)
C_EW0 = _take(128)  # aug: rows 0-63 ew0, row 64 eb0
C_PW0 = _take(128)  # aug: rows 0-63 pw0[:64], row 64 pb0, rows 65-96 pw0[64:]
C_MW0 = _take(128)  # aug: row 64 mb0, rows 65-96 mw0
C_MW3 = _take(96)   # mw3 [128,75] padded to 96 cols (dst psum [0:96])
C_EW3 = _take(3)
C_PW3 = _take(3)
NCONST = _col

# f32 bias pack (per-partition bias vectors for ACT/DVE ops)
B_EB1, B_MB1, B_PB1, B_EB2, B_MB2, B_PB2 = range(6)
NBIAS = 6


def _pack_consts(w):
    P = np.zeros((128, NCONST), ACT_NP)

    def put(cr, arr, r0=0):
        c0, c1 = cr
        a = np.asarray(arr, np.float32).astype(ACT_NP)
        P[r0 : r0 + a.shape[0], c0 : c0 + a.shape[1]] = a

    put(C_EW1, w["ew1"])
    put(C_EW2, w["ew2"])
    put(C_PW1, w["pw1"])
    put(C_PW2, w["pw2"])
    put(C_MW1, w["mw1"])
    put(C_MW2, w["mw2"])
    put(C_EW0, w["ew0"])
    put(C_EW0, w["eb0"][None, :], r0=64)
    put(C_PW0, w["pw0"][0:64])
    put(C_PW0, w["pb0"][None, :], r0=64)
    put(C_PW0, w["pw0"][64:96], r0=65)
    put(C_MW0, w["mb0"][None, :], r0=64)
    put(C_MW0, w["mw0"], r0=65)
    put(C_MW3, w["mw3"])
    put(C_EW3, w["ew3"])
    put(C_PW3, w["pw3"])
    return P


def _pack_biases(w):
    Q = np.zeros((128, NBIAS), np.float32)
    for col, key in [(B_EB1, "eb1"), (B_MB1, "mb1"), (B_PB1, "pb1"),
                     (B_EB2, "eb2"), (B_MB2, "mb2"), (B_PB2, "pb2")]:
        Q[:, col] = np.asarray(w[key], np.float32)
    return Q


def build_program():
    """Build the per-core Bass program (same SPMD program on all 8 cores)."""
    nc = bacc.Bacc("TRN2", target_bir_lowering=False, debug=False)

    sg = nc.dram_tensor("sg", [128, BC], ACT_DT, kind="ExternalInput")
    wpack = nc.dram_tensor("wpack", [128, NCONST], ACT_DT, kind="ExternalInput")
    bpack = nc.dram_tensor("bpack", [128, NBIAS], mybir.dt.float32, kind="ExternalInput")
    out_mu = nc.dram_tensor("out_mu", [75, BC], mybir.dt.float32, kind="ExternalOutput")
    out_z = nc.dram_tensor("out_z", [3, BC], mybir.dt.float32, kind="ExternalOutput")
    out_a = nc.dram_tensor("out_a", [3, BC], mybir.dt.float32, kind="ExternalOutput")

    relu = mybir.ActivationFunctionType.Relu
    copyf = mybir.ActivationFunctionType.Copy
    add_op = mybir.AluOpType.add
    max_op = mybir.AluOpType.max

    with tile.TileContext(nc) as tc:
        with (
            tc.tile_pool(name="consts", bufs=1) as consts,
            tc.tile_pool(name="ins", bufs=3) as ins,
            tc.tile_pool(name="acts", bufs=2) as acts,
            tc.tile_pool(name="outs", bufs=3) as outs,
            tc.tile_pool(name="ps", bufs=4, space="PSUM") as ps,
        ):
            W = consts.tile([128, NCONST], ACT_DT)
            nc.sync.dma_start(out=W[:], in_=wpack[:])
            BV = consts.tile([128, NBIAS], mybir.dt.float32)
            nc.sync.dma_start(out=BV[:], in_=bpack[:])

            def wv(cr, r0=0, r1=128):
                c0, c1 = cr
                return W[r0:r1, c0:c1]

            def mm(out, lhsT, rhs):
                # split into N=512 moving chunks
                n = rhs.shape[-1]
                for j in range(0, n, NMM):
                    nc.tensor.matmul(
                        out[:, j : j + NMM], lhsT, rhs[:, j : j + NMM],
                        start=True, stop=True,
                    )

            def relu_bias(key, out, in_, bcol):
                b = BV[:, bcol : bcol + 1]
                if ENG[key] == "A":
                    nc.scalar.activation(out=out, in_=in_, func=relu, bias=b)
                else:
                    nc.vector.tensor_scalar(
                        out=out, in0=in_, scalar1=b, scalar2=0.0,
                        op0=add_op, op1=max_op,
                    )

            def relu_imm(key, out, in_):
                if ENG[key] == "A":
                    nc.scalar.activation(out=out, in_=in_, func=relu)
                else:
                    nc.vector.tensor_scalar_max(out=out, in0=in_, scalar1=0.0)

            def copy(key, out, in_):
                if ENG[key] == "A":
                    nc.scalar.activation(out=out, in_=in_, func=copyf)
                else:
                    nc.vector.tensor_copy(out=out, in_=in_)

            _pn = [0]

            def psum():
                _pn[0] += 1
                return ps.tile(
                    [128, NB], mybir.dt.float32, tag="pnet", name=f"pp{_pn[0]}"
                )

            for t in range(NSLOTS):
                c0 = t * NB
                sgt = ins.tile([128, NB], ACT_DT, tag="sgt")
                nc.sync.dma_start(out=sgt[:], in_=sg[:, c0 : c0 + NB])

                # ---- layer 0 (biases folded via ones-row augmentation) ----
                p0e = psum()
                mm(p0e, wv(C_EW0, 0, 65), sgt[0:65])
                a1e = acts.tile([128, NB], ACT_DT, tag="a1e")
                relu_imm("r0e", a1e[:], p0e[:])

                p0m = psum()
                mm(p0m, wv(C_MW0, 64, 97), sgt[64:97])
                a1m = acts.tile([128, NB], ACT_DT, tag="a1m")
                relu_imm("r0m", a1m[:], p0m[:])

                p0p = psum()
                mm(p0p, wv(C_PW0, 0, 97), sgt[0:97])
                a1p = acts.tile([128, NB], ACT_DT, tag="a1p")
                relu_imm("r0p", a1p[:], p0p[:])

                # ---- layers 1 and 2 ----
                p1e = psum()
                mm(p1e, wv(C_EW1), a1e[:])
                a2e = acts.tile([128, NB], ACT_DT, tag="a2e")
                relu_bias("r1e", a2e[:], p1e[:], B_EB1)

                p1m = psum()
                mm(p1m, wv(C_MW1), a1m[:])
                a2m = acts.tile([128, NB], ACT_DT, tag="a2m")
                relu_bias("r1m", a2m[:], p1m[:], B_MB1)

                p1p = psum()
                mm(p1p, wv(C_PW1), a1p[:])
                a2p = acts.tile([128, NB], ACT_DT, tag="a2p")
                relu_bias("r1p", a2p[:], p1p[:], B_PB1)

                p2e = psum()
                mm(p2e, wv(C_EW2), a2e[:])
                a3e = acts.tile([128, NB], ACT_DT, tag="a3e")
                relu_bias("r2e", a3e[:], p2e[:], B_EB2)

                p2m = psum()
                mm(p2m, wv(C_MW2), a2m[:])
                a3m = acts.tile([128, NB], ACT_DT, tag="a3m")
                relu_bias("r2m", a3m[:], p2m[:], B_MB2)

                p2p = psum()
                mm(p2p, wv(C_PW2), a2p[:])
                a3p = acts.tile([128, NB], ACT_DT, tag="a3p")
                relu_bias("r2p", a3p[:], p2p[:], B_PB2)

                # ---- layer 3 (biases added on host) ----
                pl3 = psum()
                mm(pl3[0:96], wv(C_MW3), a3m[:])
                st3 = outs.tile([75, NB], mybir.dt.float32, tag="st3")
                copy("cmu", st3[:], pl3[0:75])
                nc.sync.dma_start(out=out_mu[:, c0 : c0 + NB], in_=st3[:])

                pz = psum()
                mm(pz[0:3], wv(C_EW3), a3e[:])
                stz = outs.tile([3, NB], mybir.dt.float32, tag="stz")
                copy("cz", stz[:], pz[0:3])
                nc.sync.dma_start(out=out_z[:, c0 : c0 + NB], in_=stz[:])

                pa = psum()
                mm(pa[0:3], wv(C_PW3), a3p[:])
                sta = outs.tile([3, NB], mybir.dt.float32, tag="sta")
                copy("ca", sta[:], pa[0:3])
                nc.sync.dma_start(out=out_a[:, c0 : c0 + NB], in_=sta[:])

    nc.compile()
    return nc


_NC = None
LAST_RESULTS = None  # BassKernelResults from the most recent run (for test.py)


def _get_nc():
    global _NC
    if _NC is None:
        _NC = build_program()
    return _NC


def kernel(**inputs):
    global LAST_RESULTS
    w = {k: np.asarray(v, np.float32) for k, v in inputs.items()}
    s, g = w["s"], w["g"]

    wpack = _pack_consts(w)
    bpack = _pack_biases(w)
    in_maps = []
    for c in range(NCORES):
        r0 = c * BC
        sgT = np.zeros((128, BC), ACT_NP)
        sgT[0:64] = s[r0 : r0 + BC].T.astype(ACT_NP)
        sgT[64] = 1.0
        sgT[65:97] = g[r0 : r0 + BC].T.astype(ACT_NP)
        in_maps.append(
            {"sg": np.ascontiguousarray(sgT), "wpack": wpack, "bpack": bpack}
        )

    nc = _get_nc()
    res = run_bass_kernel_spmd(
        nc,
        in_maps,
        core_ids=list(range(NCORES)),
        trace=bool(int(os.environ.get("KERNEL_TRACE", "0"))),
    )
    LAST_RESULTS = res

    mu = np.empty((B, MU), np.float32)
    z = np.empty((B, ADIM), np.float32)
    ail = np.empty((B, ADIM), np.float32)
    for c in range(NCORES):
        r0 = c * BC
        mu[r0 : r0 + BC] = res.results[c]["out_mu"].T
        z[r0 : r0 + BC] = res.results[c]["out_z"].T
        ail[r0 : r0 + BC] = res.results[c]["out_a"].T

    # layer-3 biases applied on host
    mu += np.asarray(w["mb3"], np.float32)[None, :]
    z += np.asarray(w["eb3"], np.float32)[None, :]
    ail += np.asarray(w["pb3"], np.float32)[None, :]

    # ---- host KDE tail + global-norm mix ----
    diff = z[:, None, :] - mu.reshape(B, NG, ADIM)  # [B, 25, 3]
    delta = -0.5 * np.einsum("bnd,bnd->bn", diff, diff) / (H * H)
    p = KDE_C * np.exp(delta)  # [B, 25]
    rho = p.sum(axis=-1)  # [B]
    grad = -np.einsum("bn,bnd->bd", p, diff) / (H * H)
    grad = np.nan_to_num(grad, nan=0.0)
    gnorm = np.linalg.norm(grad)
    gradn = grad / gnorm * NI
    pm = np.tanh(rho * 0.002)[:, None]
    out = pm * ail + (1.0 - pm) * gradn
    return out.astype(np.float32)
